# revision 1
# baseline (speedup 1.0000x reference)
"""Self-contained Trainium2 kernel for nn_Attention_56607668961538.

kernel(**inputs) takes the FULL unsharded inputs (B=16, N=1024, C=1024),
shards data-parallel over batch across 8 NeuronCores, runs a Bass/Tile
attention kernel per core via run_bass_kernel_spmd, and gathers the full
output.  See build_attention below for the on-device layout strategy.
"""

import sys

sys.path.insert(0, "/opt/trn_rl_repo")

import numpy as np

from contextlib import ExitStack

import numpy as np

import concourse.bass as bass
import concourse.mybir as mybir
import concourse.tile as tile

F32 = mybir.dt.float32
F32R = mybir.dt.float32r
EPS = 1e-6


def build_attention(nc, B_local, N, C, H, reps=1):
    Dh = C // H
    assert Dh == 64
    KT = C // 128           # contraction k-tiles
    NT = N // 128           # token m-tiles per batch
    FQK = 2 * C // 128      # q+k feature tiles
    TCH = min(512, N)       # token chunk for phase 1
    NCH = N // TCH
    VW = min(256, C)        # v-weight chunk width
    VH = VW // Dh           # heads per v chunk
    scale = Dh ** -0.5
    E = 65                  # Dh + ones column

    def halves():
        return [(off, min(512, N - off)) for off in range(0, N, 512)]

    # ---- external I/O ----
    xT = nc.dram_tensor("xT", [B_local, C, N], F32R, kind="ExternalInput").ap()
    qk_wT = nc.dram_tensor("qk_wT", [2 * C // 128, 128, C], F32R,
                           kind="ExternalInput").ap()
    v_wT = nc.dram_tensor("v_wT", [C, C], F32R, kind="ExternalInput").ap()
    proj_wT = nc.dram_tensor("proj_wT", [C // 128, 128, C], F32R,
                             kind="ExternalInput").ap()
    proj_b = nc.dram_tensor("proj_b", [C], F32, kind="ExternalInput").ap()
    mask_ssq = nc.dram_tensor("mask_ssq", [C, H], F32R, kind="ExternalInput").ap()
    w_sel = nc.dram_tensor("w_sel", [H, C], F32R, kind="ExternalInput").ap()
    sel_q = nc.dram_tensor("sel_q", [H, C], F32R, kind="ExternalInput").ap()
    den_sel = nc.dram_tensor("den_sel", [H, C], F32R, kind="ExternalInput").ap()
    vinit = nc.dram_tensor("vinit", [128, H], F32R, kind="ExternalInput").ap()
    yT = nc.dram_tensor("yT", [B_local, C, N], F32, kind="ExternalOutput").ap()

    # ---- internal DRAM ----
    qkT_d = nc.dram_tensor("qkT_d", [B_local, 2 * C, N], F32R, kind="Internal").ap()
    attn_d = nc.dram_tensor("attn_d", [B_local, C, N], F32R, kind="Internal").ap()
    ik_d = nc.dram_tensor("ik_d", [B_local, H, N], F32, kind="Internal").ap()

    with tile.TileContext(nc) as tc, ExitStack() as ctx:
        singles = ctx.enter_context(tc.tile_pool(name="singles", bufs=1))
        xp = ctx.enter_context(tc.tile_pool(name="xp", bufs=2))
        wp = ctx.enter_context(tc.tile_pool(name="wp", bufs=2))
        vwp = ctx.enter_context(tc.tile_pool(name="vwp", bufs=1))
        stagep = ctx.enter_context(tc.tile_pool(name="stagep", bufs=2))
        sqp = ctx.enter_context(tc.tile_pool(name="sqp", bufs=1))
        vainp = ctx.enter_context(tc.tile_pool(name="vainp", bufs=NT + 1))
        statp = ctx.enter_context(tc.tile_pool(name="statp", bufs=1))
        pairp = ctx.enter_context(tc.tile_pool(name="pairp", bufs=2))
        ptp = ctx.enter_context(tc.tile_pool(name="ptp", bufs=2))
        aop = ctx.enter_context(tc.tile_pool(name="aop", bufs=1))
        atnp = ctx.enter_context(tc.tile_pool(name="atnp", bufs=KT))
        pwp = ctx.enter_context(tc.tile_pool(name="pwp", bufs=2))
        ystp = ctx.enter_context(tc.tile_pool(name="ystp", bufs=2))

        mmps = ctx.enter_context(tc.tile_pool(name="mmps", bufs=2, space="PSUM"))
        stps = ctx.enter_context(tc.tile_pool(name="stps", bufs=2, space="PSUM"))
        avps = ctx.enter_context(tc.tile_pool(name="avps", bufs=1, space="PSUM"))

        # ---- constants ----
        mask_sb = singles.tile([128, KT, H], F32R)
        nc.sync.dma_start(out=mask_sb, in_=mask_ssq.rearrange("(k p) h -> p k h", p=128))
        wsel_sb = singles.tile([H, C], F32R)
        nc.sync.dma_start(out=wsel_sb, in_=w_sel)
        selq_sb = singles.tile([H, C], F32R)
        nc.sync.dma_start(out=selq_sb, in_=sel_q)
        densel_sb = singles.tile([H, C], F32R)
        nc.sync.dma_start(out=densel_sb, in_=den_sel)
        bias_sb = singles.tile([128, KT], F32)
        nc.sync.dma_start(out=bias_sb, in_=proj_b.rearrange("(k p) -> p k", p=128))
        eps_sb = singles.tile([H, 1], F32)
        nc.vector.memset(eps_sb, EPS)

        loop = ctx.enter_context(tc.For_i(0, reps, 1)) if reps > 1 else None
        for b in range(B_local):
            # ================= phase 1: qkv projection + ssq stats ==========
            ssq_q = statp.tile([H, N], F32, tag="ssqq")
            ssq_k = statp.tile([H, N], F32, tag="ssqk")
            x_sb = {}
            for tcn in range(NCH):
                xt = xp.tile([128, KT, TCH], F32R, tag="x")
                nc.gpsimd.dma_start(
                    out=xt,
                    in_=xT[b, :, tcn * TCH:(tcn + 1) * TCH].rearrange(
                        "(k p) t -> p k t", p=128))
                x_sb[tcn] = xt
                tsl = slice(tcn * TCH, (tcn + 1) * TCH)
                for ft in range(FQK):
                    wt = wp.tile([128, KT, 128], F32R, tag="qkw")
                    nc.gpsimd.dma_start(
                        out=wt, in_=qk_wT[ft].rearrange("p (k f) -> p k f", f=128))
                    ps = mmps.tile([128, TCH], F32, tag="mm")
                    for k in range(KT):
                        nc.tensor.matmul(ps, wt[:, k], xt[:, k],
                                         start=(k == 0), stop=(k == KT - 1))
                    st = stagep.tile([128, TCH], F32R, tag="stage")
                    nc.vector.tensor_copy(st, ps)
                    nc.sync.dma_start(out=qkT_d[b, ft * 128:(ft + 1) * 128, tsl],
                                      in_=st)
                    sq = sqp.tile([128, TCH], F32R, tag="sq")
                    nc.vector.tensor_mul(sq, st, st)
                    ps2 = mmps.tile([128, TCH], F32, tag="mm")
                    nc.tensor.matmul(ps2[:H], mask_sb[:, ft % KT], sq,
                                     start=True, stop=True)
                    acc = ssq_q if ft < KT else ssq_k
                    if ft % KT == 0:
                        nc.vector.tensor_copy(acc[:, tsl], ps2[:H])
                    else:
                        nc.vector.tensor_add(acc[:, tsl], acc[:, tsl], ps2[:H])

            # V part: token-major into persistent augmented SBUF tiles
            va_t = {}
            for vc in range(C // VW):
                vwt = vwp.tile([128, KT, VW], F32R, tag="vw")
                nc.gpsimd.dma_start(
                    out=vwt,
                    in_=v_wT[:, vc * VW:(vc + 1) * VW].rearrange(
                        "(k p) f -> p k f", p=128))
                for tcn in range(NCH):
                    for tm in range(TCH // 128):
                        j = tcn * (TCH // 128) + tm
                        if vc == 0:
                            va_t[j] = vainp.tile([128, H, E], F32R, tag="vain",
                                                 name=f"vain_{b}_{j}")
                            nc.sync.dma_start(out=va_t[j][:, :, 64:65],
                                              in_=vinit.unsqueeze(-1))
                        ps = mmps.tile([128, VW], F32, tag="mm")
                        for k in range(KT):
                            nc.tensor.matmul(
                                ps, x_sb[tcn][:, k, tm * 128:(tm + 1) * 128],
                                vwt[:, k], start=(k == 0), stop=(k == KT - 1))
                        nc.vector.tensor_copy(
                            va_t[j][:, vc * VH:(vc + 1) * VH, 0:64],
                            ps.rearrange("p (h e) -> p h e", e=64))

            # ================= rmsnorm stats tail ===========================
            invr = {}
            for nm, acc in (("q", ssq_q), ("k", ssq_k)):
                ivr = statp.tile([H, N], F32R, tag="invr" + nm)
                for off, w in halves():
                    rms = statp.tile([H, TCH], F32, tag="rms")
                    nc.scalar.activation(out=rms[:, :w], in_=acc[:, off:off + w],
                                         func=mybir.ActivationFunctionType.Sqrt,
                                         bias=eps_sb, scale=1.0 / Dh)
                    inv = statp.tile([H, TCH], F32, tag="inv")
                    nc.vector.reciprocal_approx_fast(out=inv[:, :w],
                                                     in_=rms[:, :w])
                    nc.vector.tensor_copy(ivr[:, off:off + w], inv[:, :w])
                invr[nm] = ivr

            # token-major invr_k for the exp per-partition scale:
            # ikT[p, j, h] = invr_k[h, j*128 + p]
            ikT = statp.tile([128, NT, H], F32, tag="ikT")
            nc.sync.dma_start(out=ik_d[b], in_=invr["k"].bitcast(F32))
            for h in range(H):
                nc.sync.dma_start(
                    out=ikT[:, :, h],
                    in_=ik_d[b, h].rearrange("(j q) -> q j", q=128))

            # ================= attention ====================================
            den_all = statp.tile([H, N], F32, tag="den")
            for h in range(H):
                qt = pairp.tile([64, N], F32R, tag="qt")
                nc.sync.dma_start(out=qt, in_=qkT_d[b, h * 64:(h + 1) * 64, :])
                kt = pairp.tile([64, N], F32R, tag="kt")
                nc.sync.dma_start(out=kt,
                                  in_=qkT_d[b, C + h * 64:C + (h + 1) * 64, :])
                for off, w in halves():
                    sl = slice(off, off + w)
                    bq = mmps.tile([128, TCH], F32, tag="mm")
                    nc.tensor.matmul(bq[:64, :w], selq_sb[:, h * 64:(h + 1) * 64],
                                     invr["q"][:, sl], start=True, stop=True)
                    nc.vector.tensor_mul(qt[:, sl], qt[:, sl], bq[:64, :w])

                av = avps.tile([128, N], F32, tag="av")
                for j in range(NT):
                    va = va_t[j][:, h, :]
                    st_ps = stps.tile([128, N], F32, tag="st")
                    for off, w in halves():
                        nc.tensor.matmul(st_ps[:, off:off + w],
                                         kt[:, j * 128:(j + 1) * 128],
                                         qt[:, off:off + w],
                                         start=True, stop=True)
                    pt = ptp.tile([128, N], F32R, tag="pt")
                    nc.scalar.activation(out=pt, in_=st_ps,
                                         func=mybir.ActivationFunctionType.Exp,
                                         scale=ikT[:, j, h].unsqueeze(-1))
                    for off, w in halves():
                        nc.tensor.matmul(av[0:E, off:off + w], va,
                                         pt[:, off:off + w],
                                         start=(j == 0), stop=(j == NT - 1))

                ao = aop.tile([E, N], F32R, tag="ao")
                nc.vector.tensor_copy(ao, av[0:E, :])
                nc.sync.dma_start(out=attn_d[b, h * 64:(h + 1) * 64, :], in_=ao[0:64])
                nc.sync.dma_start(out=den_all[h:h + 1, :],
                                  in_=ao[64:65, :].bitcast(F32))
            del va

            invden = statp.tile([H, N], F32R, tag="invden")
            for off, w in halves():
                dtmp = statp.tile([H, TCH], F32, tag="inv")
                nc.vector.reciprocal_approx_fast(out=dtmp[:, :w],
                                                 in_=den_all[:, off:off + w])
                nc.vector.tensor_copy(invden[:, off:off + w], dtmp[:, :w])

            # ================= projection (per token-half) ==================
            for off, w in halves():
                sl = slice(off, off + w)
                atn = []
                for k in range(KT):
                    raw = stagep.tile([128, TCH], F32R, tag="rawa")
                    nc.sync.dma_start(out=raw[:, :w],
                                      in_=attn_d[b, k * 128:(k + 1) * 128, sl])
                    at = atnp.tile([128, TCH], F32R, tag="atn")
                    bd = mmps.tile([128, TCH], F32, tag="mm")
                    nc.tensor.matmul(bd[:, :w], densel_sb[:, k * 128:(k + 1) * 128],
                                     invden[:, sl], start=True, stop=True)
                    nc.vector.tensor_mul(at[:, :w], raw[:, :w], bd[:, :w])
                    atn.append(at)

                for mt in range(KT):
                    pw = pwp.tile([128, KT, 128], F32R, tag="pw")
                    nc.gpsimd.dma_start(
                        out=pw, in_=proj_wT[mt].rearrange("p (k f) -> p k f", f=128))
                    ps = mmps.tile([128, TCH], F32, tag="mm")
                    for k in range(KT):
                        nc.tensor.matmul(ps[:, :w], pw[:, k], atn[k][:, :w],
                                         start=(k == 0), stop=(k == KT - 1))
                    yst = ystp.tile([128, TCH], F32, tag="yst")
                    nc.vector.tensor_scalar_add(yst[:, :w], ps[:, :w],
                                                bias_sb[:, mt:mt + 1])
                    nc.sync.dma_start(
                        out=yT[b, mt * 128:(mt + 1) * 128, sl],
                        in_=yst[:, :w])
                del atn

    return nc


def prep_inputs(x, qkv_w, proj_w, proj_b, q_norm_w, k_norm_w, n_cores):
    """Host-side prep: shard over batch, pre-transpose, build selector masks.
    Returns (in_maps, meta) where in_maps[i] is the input dict for core i."""
    B, N, C = x.shape
    H = C // 64
    Dh = 64
    B_local = B // n_cores
    scale = Dh ** -0.5

    qkv_wT = np.ascontiguousarray(qkv_w.T)          # [C, 3C]
    # per-Mtile contiguous layout [ft, p, k*128+f] so weight DMAs are linear
    qk_wT = np.ascontiguousarray(
        qkv_wT[:, :2 * C].reshape(C // 128, 128, 2 * C // 128, 128)
        .transpose(2, 1, 0, 3).reshape(2 * C // 128, 128, C))
    v_wT = np.ascontiguousarray(qkv_wT[:, 2 * C:])
    proj_wT = np.ascontiguousarray(
        proj_w.T.reshape(C // 128, 128, C // 128, 128)
        .transpose(2, 1, 0, 3).reshape(C // 128, 128, C))

    heads = np.arange(C) // Dh                       # head index per channel
    mask_ssq = (heads[:, None] == np.arange(H)[None, :]).astype(np.float32)
    w_qk = (q_norm_w * k_norm_w).astype(np.float32)  # [Dh]
    w_sel = np.zeros((H, C), np.float32)
    sel_q = np.zeros((H, C), np.float32)
    den_sel = np.zeros((H, C), np.float32)
    for h in range(H):
        w_sel[h, h * Dh:(h + 1) * Dh] = w_qk
        sel_q[h, h * Dh:(h + 1) * Dh] = scale * w_qk
        den_sel[h, h * Dh:(h + 1) * Dh] = 1.0

    shared = dict(qk_wT=qk_wT, v_wT=v_wT, proj_wT=proj_wT,
                  proj_b=proj_b.astype(np.float32), mask_ssq=mask_ssq,
                  w_sel=w_sel, sel_q=sel_q, den_sel=den_sel,
                  vinit=np.ones((128, H), np.float32))
    in_maps = []
    for i in range(n_cores):
        xs = x[i * B_local:(i + 1) * B_local]        # [B_local, N, C]
        xTl = np.ascontiguousarray(xs.transpose(0, 2, 1))  # [B_local, C, N]
        in_maps.append(dict(xT=xTl, **shared))
    return in_maps, dict(B=B, N=N, C=C, H=H, B_local=B_local)


def gather_output(results, meta):
    B, N, C, B_local = meta["B"], meta["N"], meta["C"], meta["B_local"]
    y = np.empty((B, N, C), np.float32)
    for i, r in enumerate(results):
        yTl = r["yT"]                                # [B_local, C, N]
        y[i * B_local:(i + 1) * B_local] = yTl.transpose(0, 2, 1)
    return y


N_CORES = 8
_CACHE = {}


def _get_nc():
    if "nc" not in _CACHE:
        from concourse import bacc

        nc = bacc.Bacc("TRN2", target_bir_lowering=False, debug=False,
                       num_devices=N_CORES)
        build_attention(nc, B_local=16 // N_CORES, N=1024, C=1024, H=16)
        nc.compile()
        _CACHE["nc"] = nc
    return _CACHE["nc"]


def run_sharded(in_maps, trace=False):
    from concourse.bass_utils import run_bass_kernel_spmd

    return run_bass_kernel_spmd(_get_nc(), in_maps,
                                core_ids=list(range(N_CORES)), trace=trace)


def kernel(x, qkv_w, proj_w, proj_b, q_norm_w, k_norm_w):
    x = np.asarray(x)
    in_maps, meta = prep_inputs(np.asarray(x), np.asarray(qkv_w),
                                np.asarray(proj_w), np.asarray(proj_b),
                                np.asarray(q_norm_w), np.asarray(k_norm_w),
                                N_CORES)
    res = run_sharded(in_maps)
    return gather_output(res.results, meta)



# revision 12
# speedup vs baseline: 1.2754x; 1.2754x over previous
"""Self-contained Trainium2 kernel for nn_Attention_56607668961538.

kernel(**inputs) takes the FULL unsharded inputs (B=16, N=1024, C=1024),
shards data-parallel over batch across 8 NeuronCores (B_local=2), runs a
Bass/Tile attention kernel per core via run_bass_kernel_spmd, and gathers
the full output.

v2 design (vs the fp32r/DRAM-round-trip baseline):
  - all matmul operands bf16 (inputs pre-cast on host); fp32 PSUM accum
  - zero DRAM round-trips: q/k/v/attn all SBUF-resident per batch item
  - rmsnorm scales (q_norm*k_norm*softmax scale, inv_rms) folded into
    q/k tiles at projection-evacuation time via PE-broadcast + DVE mul
  - attention runs per head-PAIR (the two heads sharing a 128-partition
    feature tile), 512-query chunks: QK^T -> one paired exp[128,1024]
    on ScalarE -> AV with an appended ones-column producing the softmax
    denominator; denominator reciprocal on VectorE; normalization folded
    into the output-projection phase via a second PE-broadcast
  - PSUM budget exactly 8 banks: scores 2x[128,1024]f32, av 2x[65,512]f32,
    generic mm 2x[128,512]f32
  - PSUM->SBUF evacuations ride the Scalar engine (Copy) where VectorE
    is needed elsewhere, keeping TensorE streams dense (HAM warm)
"""

import sys

sys.path.insert(0, "/opt/trn_rl_repo")

from contextlib import ExitStack

import numpy as np

import concourse.bass as bass
import concourse.mybir as mybir
import concourse.tile as tile

F32 = mybir.dt.float32
F32R = mybir.dt.float32r
BF16 = mybir.dt.bfloat16
EPS = 1e-6
AF = mybir.ActivationFunctionType


def build_attention(nc, B_local, N, C, H, reps=1, dbg=False):
    Dh = C // H
    assert Dh == 64 and N == 1024 and C == 1024 and H == 16
    KT = C // 128            # 8 contraction k-tiles
    FT = C // 128            # 8 feature tiles per tensor (q, k, v, attn)
    NP = 8                   # head pairs
    QC = 512                 # query chunk (fp32 PSUM bank limit)

    # ---- external I/O ----
    xT = nc.dram_tensor("xT", [B_local, C, N], BF16, kind="ExternalInput").ap()
    qk_wT = nc.dram_tensor("qk_wT", [2 * FT, 128, C], BF16,
                           kind="ExternalInput").ap()
    v_wT = nc.dram_tensor("v_wT", [C, C], BF16, kind="ExternalInput").ap()
    proj_wT = nc.dram_tensor("proj_wT", [FT, 128, C], BF16,
                             kind="ExternalInput").ap()
    proj_b = nc.dram_tensor("proj_b", [C], F32, kind="ExternalInput").ap()
    mask_ssq = nc.dram_tensor("mask_ssq", [128, FT, H], BF16,
                              kind="ExternalInput").ap()
    sel_q = nc.dram_tensor("sel_q", [H, C], BF16, kind="ExternalInput").ap()
    sel_k = nc.dram_tensor("sel_k", [H, C], BF16, kind="ExternalInput").ap()
    vinit = nc.dram_tensor("vinit", [128, H], BF16, kind="ExternalInput").ap()
    yT = nc.dram_tensor("yT", [B_local, C, N], BF16, kind="ExternalOutput").ap()
    if dbg:
        dbg_q = nc.dram_tensor("dbg_q", [C, N], BF16, kind="ExternalOutput").ap()
        dbg_k = nc.dram_tensor("dbg_k", [C, N], BF16, kind="ExternalOutput").ap()
        dbg_araw = nc.dram_tensor("dbg_araw", [C, N], BF16,
                                  kind="ExternalOutput").ap()
        dbg_invden = nc.dram_tensor("dbg_invden", [H, N], BF16,
                                    kind="ExternalOutput").ap()
        dbg_va = nc.dram_tensor("dbg_va", [128, H, 66], BF16,
                                kind="ExternalOutput").ap()
        dbg_dens = nc.dram_tensor("dbg_dens", [8, 2, 2, 512], F32,
                                  kind="ExternalOutput").ap()
        dbg_dens2 = nc.dram_tensor("dbg_dens2", [8, 2, 2, 512], F32,
                                   kind="ExternalOutput").ap()

    with tile.TileContext(nc) as tc, ExitStack() as ctx:
        singles = ctx.enter_context(tc.tile_pool(name="singles", bufs=1))
        xp = ctx.enter_context(tc.tile_pool(name="xp", bufs=1))
        wp = ctx.enter_context(tc.tile_pool(name="wp", bufs=2))
        qkp = ctx.enter_context(tc.tile_pool(name="qkp", bufs=2 * FT))
        sqp = ctx.enter_context(tc.tile_pool(name="sqp", bufs=1))
        vap = ctx.enter_context(tc.tile_pool(name="vap", bufs=2 * KT))
        ptp = ctx.enter_context(tc.tile_pool(name="ptp", bufs=2))
        aop = ctx.enter_context(tc.tile_pool(name="aop", bufs=2 * FT))
        statp = ctx.enter_context(tc.tile_pool(name="statp", bufs=2))
        ystp = ctx.enter_context(tc.tile_pool(name="ystp", bufs=2))

        mmps = ctx.enter_context(tc.tile_pool(name="mmps", bufs=2, space="PSUM"))
        stps = ctx.enter_context(tc.tile_pool(name="stps", bufs=2, space="PSUM"))
        avps = ctx.enter_context(tc.tile_pool(name="avps", bufs=2, space="PSUM"))

        # ---- constants / weights resident across items ----
        mask_sb = singles.tile([128, FT, H], BF16)
        nc.sync.dma_start(out=mask_sb, in_=mask_ssq)
        selq_sb = singles.tile([H, C], BF16)
        nc.sync.dma_start(out=selq_sb, in_=sel_q)
        selk_sb = singles.tile([H, C], BF16)
        nc.sync.dma_start(out=selk_sb, in_=sel_k)
        bias_sb = singles.tile([128, KT], F32)
        nc.sync.dma_start(out=bias_sb, in_=proj_b.rearrange("(k p) -> p k", p=128))
        eps_sb = singles.tile([H, 1], F32)
        nc.vector.memset(eps_sb, EPS)
        vw_sb = singles.tile([128, KT, C], BF16)
        nc.gpsimd.dma_start(out=vw_sb,
                            in_=v_wT.rearrange("(k p) f -> p k f", p=128))

        bsp = ctx.enter_context(tc.tile_pool(name="bsp", bufs=2))
        dsp = ctx.enter_context(tc.tile_pool(name="dsp", bufs=1))

        loop = ctx.enter_context(tc.For_i(0, reps, 1)) if reps > 1 else None
        for b in range(B_local):
            # ============ phase A: qkv projection + rmsnorm stats ==========
            x_sb = xp.tile([128, KT, N], BF16, tag="x", name=f"x_{b}")
            nc.gpsimd.dma_start(
                out=x_sb, in_=xT[b].rearrange("(k p) t -> p k t", p=128))

            ssq_q = statp.tile([H, N], F32, tag="ssqq")
            ssq_k = statp.tile([H, N], F32, tag="ssqk")
            q_sb, k_sb = {}, {}
            for ft in range(2 * FT):
                f = ft % FT
                wt = wp.tile([128, C], BF16, tag="qkw")
                nc.gpsimd.dma_start(out=wt, in_=qk_wT[ft])
                dstmap = q_sb if ft < FT else k_sb
                dst = qkp.tile([128, N], BF16, tag="qk",
                               name=f"{'q' if ft < FT else 'k'}_{b}_{f}")
                dstmap[f] = dst
                sqt = sqp.tile([128, N], BF16, tag="sq")
                acc = ssq_q if ft < FT else ssq_k
                for half in range(2):
                    tsl = slice(half * QC, (half + 1) * QC)
                    ps = mmps.tile([128, QC], F32, tag="mm")
                    for k in range(KT):
                        nc.tensor.matmul(ps, wt[:, k * 128:(k + 1) * 128],
                                         x_sb[:, k, tsl],
                                         start=(k == 0), stop=(k == KT - 1))
                    # evacuate on ScalarE so the mm slot frees fast
                    nc.scalar.activation(out=dst[:, tsl], in_=ps, func=AF.Copy)
                    nc.vector.tensor_mul(sqt[:, tsl], dst[:, tsl], dst[:, tsl])
                    ps2 = mmps.tile([128, QC], F32, tag="mm")
                    nc.tensor.matmul(ps2[:H], mask_sb[:, f], sqt[:, tsl],
                                     start=True, stop=True)
                    if f == 0:
                        nc.vector.tensor_copy(acc[:, tsl], ps2[:H])
                    else:
                        nc.vector.tensor_add(acc[:, tsl], acc[:, tsl], ps2[:H])

            # rmsnorm: invr = 1/sqrt(ssq/Dh + eps), per tensor
            invr = {}
            for nm, acc in (("q", ssq_q), ("k", ssq_k)):
                nc.scalar.activation(out=acc, in_=acc, func=AF.Sqrt,
                                     bias=eps_sb, scale=1.0 / Dh)
                nc.vector.reciprocal_approx_fast(out=acc, in_=acc)
                ivr = statp.tile([H, N], BF16, tag="invr" + nm)
                nc.vector.tensor_copy(ivr, acc)
                invr[nm] = ivr

            # scale pass: q *= bcast(selq^T invr_q); k *= bcast(selk^T invr_k)
            for ft in range(2 * FT):
                f = ft % FT
                sel = selq_sb if ft < FT else selk_sb
                ivr = invr["q"] if ft < FT else invr["k"]
                dst = (q_sb if ft < FT else k_sb)[f]
                fsl = slice(f * 128, (f + 1) * 128)
                for half in range(2):
                    tsl = slice(half * QC, (half + 1) * QC)
                    bc = mmps.tile([128, QC], F32, tag="mm")
                    nc.tensor.matmul(bc, sel[:, fsl], ivr[:, tsl],
                                     start=True, stop=True)
                    nc.vector.tensor_mul(dst[:, tsl], dst[:, tsl], bc)

            if dbg and b == 0:
                for f in range(FT):
                    nc.sync.dma_start(out=dbg_q[f * 128:(f + 1) * 128, :],
                                      in_=q_sb[f])
                    nc.sync.dma_start(out=dbg_k[f * 128:(f + 1) * 128, :],
                                      in_=k_sb[f])

            # V projection: token-major augmented tiles [tok, head, Dh+1]
            va = {}
            for j in range(KT):
                vat = vap.tile([128, H, 66], BF16, tag="va", name=f"va_{b}_{j}")
                va[j] = vat
                nc.sync.dma_start(out=vat[:, :, 64:65], in_=vinit.unsqueeze(-1))
                for half in range(2):
                    vsl = slice(half * QC, (half + 1) * QC)
                    ps = mmps.tile([128, QC], F32, tag="mm")
                    for k in range(KT):
                        nc.tensor.matmul(ps, x_sb[:, k, j * 128:(j + 1) * 128],
                                         vw_sb[:, k, vsl],
                                         start=(k == 0), stop=(k == KT - 1))
                    nc.scalar.activation(
                        out=vat[:, half * FT:(half + 1) * FT, 0:64],
                        in_=ps.rearrange("p (h e) -> p h e", e=64),
                        func=AF.Copy)

            # ============ phase B: attention per head pair =================
            invden = statp.tile([H, N], BF16, tag="invden")
            den_all = statp.tile([H, N], F32, tag="denall")
            attn_sb = {}
            for p in range(NP):
                attn_sb[p] = aop.tile([128, N], BF16, tag="attn",
                                      name=f"attn_{b}_{p}")
            for p in range(NP):
                kA = k_sb[p][0:64]
                kB = k_sb[p][64:128]
                qA = q_sb[p][0:64]
                qB = q_sb[p][64:128]
                for qc in range(2):
                    qsl = slice(qc * QC, (qc + 1) * QC)
                    avA = avps.tile([65, QC], F32, tag="av")
                    avB = avps.tile([65, QC], F32, tag="av")
                    for j in range(KT):
                        jsl = slice(j * 128, (j + 1) * 128)
                        st = stps.tile([128, 2 * QC], F32, tag="st")
                        nc.tensor.matmul(st[:, 0:QC], kA[:, jsl], qA[:, qsl],
                                         start=True, stop=True)
                        nc.tensor.matmul(st[:, QC:2 * QC], kB[:, jsl],
                                         qB[:, qsl], start=True, stop=True)
                        pt = ptp.tile([128, 2 * QC], BF16, tag="pt")
                        nc.scalar.activation(out=pt, in_=st, func=AF.Exp)
                        nc.tensor.matmul(avA, va[j][:, 2 * p, 0:65],
                                         pt[:, 0:QC],
                                         start=(j == 0), stop=(j == KT - 1))
                        nc.tensor.matmul(avB, va[j][:, 2 * p + 1, 0:65],
                                         pt[:, QC:2 * QC],
                                         start=(j == 0), stop=(j == KT - 1))
                    # head A features: partitions already aligned
                    nc.scalar.activation(out=attn_sb[p][0:64, qsl],
                                         in_=avA[0:64], func=AF.Copy)
                    # head B features: partition-hop 0-63 -> 64-127 via DMA
                    bst = bsp.tile([64, QC], BF16, tag="bst")
                    nc.vector.tensor_copy(bst, avB[0:64])
                    nc.sync.dma_start(out=attn_sb[p][64:128, qsl], in_=bst)
                    # denominators: reciprocal at p64, tiny DMA into [H, N]
                    # stage raw dens in SBUF, gather to [H, N] via DMA;
                    # reciprocal runs once per item at partition base 0
                    dst_den = dsp.tile([128, 2, QC], F32, tag="denst")
                    nc.vector.tensor_copy(dst_den[64:65, 0], avA[64:65])
                    nc.vector.tensor_copy(dst_den[64:65, 1], avB[64:65])
                    nc.sync.dma_start(out=den_all[2 * p:2 * p + 1, qsl],
                                      in_=dst_den[64:65, 0])
                    nc.sync.dma_start(out=den_all[2 * p + 1:2 * p + 2, qsl],
                                      in_=dst_den[64:65, 1])
                    if dbg and b == 0:
                        nc.sync.dma_start(out=dbg_dens[p, qc].unsqueeze(0),
                                          in_=dst_den[64:65])
                        dact = dsp.tile([128, 2, QC], F32, tag="denact")
                        nc.scalar.activation(out=dact[64:65, 0],
                                             in_=avA[64:65], func=AF.Copy)
                        nc.scalar.activation(out=dact[64:65, 1],
                                             in_=avB[64:65], func=AF.Copy)
                        nc.sync.dma_start(out=dbg_dens2[p, qc].unsqueeze(0),
                                          in_=dact[64:65])

            if dbg and b == 0:
                nc.sync.dma_start(out=dbg_va, in_=va[0])
                nc.sync.dma_start(out=dbg_invden, in_=invden)
                for p in range(NP):
                    nc.sync.dma_start(out=dbg_araw[p * 128:(p + 1) * 128, :],
                                      in_=attn_sb[p])

            # invden = 1/den (reuse the dead ssq_q tile as f32 scratch)
            nc.vector.reciprocal_approx_fast(out=ssq_q, in_=den_all)
            nc.vector.tensor_copy(invden, ssq_q)

            # ============ phase C: normalize + output projection ===========
            for kf in range(FT):
                fsl = slice(kf * 128, (kf + 1) * 128)
                for half in range(2):
                    tsl = slice(half * QC, (half + 1) * QC)
                    bc = mmps.tile([128, QC], F32, tag="mm")
                    nc.tensor.matmul(bc, selk_sb[:, fsl], invden[:, tsl],
                                     start=True, stop=True)
                    nc.vector.tensor_mul(attn_sb[kf][:, tsl],
                                         attn_sb[kf][:, tsl], bc)
            for mt in range(FT):
                pw = wp.tile([128, C], BF16, tag="pw")
                nc.gpsimd.dma_start(out=pw, in_=proj_wT[mt])
                yst = ystp.tile([128, N], BF16, tag="yst")
                for half in range(2):
                    tsl = slice(half * QC, (half + 1) * QC)
                    ps = mmps.tile([128, QC], F32, tag="mm")
                    for k in range(KT):
                        nc.tensor.matmul(ps, pw[:, k * 128:(k + 1) * 128],
                                         attn_sb[k][:, tsl],
                                         start=(k == 0), stop=(k == KT - 1))
                    nc.vector.tensor_scalar_add(yst[:, tsl], ps,
                                                bias_sb[:, mt:mt + 1])
                nc.sync.dma_start(out=yT[b, mt * 128:(mt + 1) * 128, :],
                                  in_=yst)

    return nc


def prep_inputs(x, qkv_w, proj_w, proj_b, q_norm_w, k_norm_w, n_cores):
    """Host-side prep: shard over batch, pre-transpose, pre-cast bf16,
    build selector masks. Returns (in_maps, meta)."""
    import ml_dtypes

    BF = ml_dtypes.bfloat16
    B, N, C = x.shape
    H = C // 64
    Dh = 64
    B_local = B // n_cores
    FT = C // 128
    scale = Dh ** -0.5

    qkv_wT = np.ascontiguousarray(qkv_w.T)           # [C, 3C]
    qk_wT = np.ascontiguousarray(
        qkv_wT[:, :2 * C].reshape(C // 128, 128, 2 * C // 128, 128)
        .transpose(2, 1, 0, 3).reshape(2 * C // 128, 128, C)).astype(BF)
    v_wT = np.ascontiguousarray(qkv_wT[:, 2 * C:]).astype(BF)
    proj_wT = np.ascontiguousarray(
        proj_w.T.reshape(C // 128, 128, C // 128, 128)
        .transpose(2, 1, 0, 3).reshape(C // 128, 128, C)).astype(BF)

    # mask_sb[p, f, h] = 1 iff feature f*128+p belongs to head h (per tensor)
    feat = np.arange(C).reshape(FT, 128)             # [f, p]
    mask_ssq = np.ascontiguousarray(
        (feat.T[:, :, None] // Dh == np.arange(H)[None, None, :])
        .astype(BF))                                 # [128, FT, H]

    w_qk = (q_norm_w * k_norm_w).astype(np.float32)  # [Dh]
    heads = np.arange(C) // Dh
    sel_q = np.zeros((H, C), np.float32)
    sel_k = np.zeros((H, C), np.float32)
    for h in range(H):
        sel_q[h, h * Dh:(h + 1) * Dh] = scale * w_qk
        sel_k[h, h * Dh:(h + 1) * Dh] = 1.0

    shared = dict(qk_wT=qk_wT, v_wT=v_wT, proj_wT=proj_wT,
                  proj_b=proj_b.astype(np.float32), mask_ssq=mask_ssq,
                  sel_q=sel_q.astype(BF), sel_k=sel_k.astype(BF),
                  vinit=np.ones((128, H), BF))
    in_maps = []
    for i in range(n_cores):
        xs = x[i * B_local:(i + 1) * B_local]        # [B_local, N, C]
        xTl = np.ascontiguousarray(xs.transpose(0, 2, 1)).astype(BF)
        in_maps.append(dict(xT=xTl, **shared))
    return in_maps, dict(B=B, N=N, C=C, H=H, B_local=B_local)


def gather_output(results, meta):
    B, N, C, B_local = meta["B"], meta["N"], meta["C"], meta["B_local"]
    y = np.empty((B, N, C), np.float32)
    for i, r in enumerate(results):
        yTl = np.asarray(r["yT"], dtype=np.float32)  # [B_local, C, N]
        y[i * B_local:(i + 1) * B_local] = yTl.transpose(0, 2, 1)
    return y


N_CORES = 8
_CACHE = {}


def _get_nc():
    if "nc" not in _CACHE:
        from concourse import bacc

        nc = bacc.Bacc("TRN2", target_bir_lowering=False, debug=False,
                       num_devices=N_CORES)
        build_attention(nc, B_local=16 // N_CORES, N=1024, C=1024, H=16)
        nc.compile()
        _CACHE["nc"] = nc
    return _CACHE["nc"]


def run_sharded(in_maps, trace=False):
    from concourse.bass_utils import run_bass_kernel_spmd

    return run_bass_kernel_spmd(_get_nc(), in_maps,
                                core_ids=list(range(N_CORES)), trace=trace)


def kernel(x, qkv_w, proj_w, proj_b, q_norm_w, k_norm_w):
    in_maps, meta = prep_inputs(np.asarray(x), np.asarray(qkv_w),
                                np.asarray(proj_w), np.asarray(proj_b),
                                np.asarray(q_norm_w), np.asarray(k_norm_w),
                                N_CORES)
    res = run_sharded(in_maps)
    return gather_output(res.results, meta)


# revision 13
# speedup vs baseline: 1.5016x; 1.1774x over previous
"""Self-contained Trainium2 kernel for nn_Attention_56607668961538.

kernel(**inputs) takes the FULL unsharded inputs (B=16, N=1024, C=1024),
shards data-parallel over batch across 8 NeuronCores (B_local=2), runs a
Bass/Tile attention kernel per core via run_bass_kernel_spmd, and gathers
the full output.

v2 design (vs the fp32r/DRAM-round-trip baseline):
  - all matmul operands bf16 (inputs pre-cast on host); fp32 PSUM accum
  - zero DRAM round-trips: q/k/v/attn all SBUF-resident per batch item
  - rmsnorm scales (q_norm*k_norm*softmax scale, inv_rms) folded into
    q/k tiles at projection-evacuation time via PE-broadcast + DVE mul
  - attention runs per head-PAIR (the two heads sharing a 128-partition
    feature tile), 512-query chunks: QK^T -> one paired exp[128,1024]
    on ScalarE -> AV with an appended ones-column producing the softmax
    denominator; denominator reciprocal on VectorE; normalization folded
    into the output-projection phase via a second PE-broadcast
  - PSUM budget exactly 8 banks: scores 2x[128,1024]f32, av 2x[65,512]f32,
    generic mm 2x[128,512]f32
  - PSUM->SBUF evacuations ride the Scalar engine (Copy) where VectorE
    is needed elsewhere, keeping TensorE streams dense (HAM warm)
"""

import sys

sys.path.insert(0, "/opt/trn_rl_repo")

from contextlib import ExitStack

import numpy as np

import concourse.bass as bass
import concourse.mybir as mybir
import concourse.tile as tile

F32 = mybir.dt.float32
F32R = mybir.dt.float32r
BF16 = mybir.dt.bfloat16
EPS = 1e-6
AF = mybir.ActivationFunctionType


def build_attention(nc, B_local, N, C, H, reps=1, dbg=False):
    Dh = C // H
    assert Dh == 64 and N == 1024 and C == 1024 and H == 16
    KT = C // 128            # 8 contraction k-tiles
    FT = C // 128            # 8 feature tiles per tensor (q, k, v, attn)
    NP = 8                   # head pairs
    QC = 512                 # query chunk (fp32 PSUM bank limit)

    # ---- external I/O ----
    xT = nc.dram_tensor("xT", [B_local, C, N], BF16, kind="ExternalInput").ap()
    qk_wT = nc.dram_tensor("qk_wT", [2 * FT, 128, C], BF16,
                           kind="ExternalInput").ap()
    v_wT = nc.dram_tensor("v_wT", [C, C], BF16, kind="ExternalInput").ap()
    proj_wT = nc.dram_tensor("proj_wT", [FT, 128, C], BF16,
                             kind="ExternalInput").ap()
    proj_b = nc.dram_tensor("proj_b", [C], F32, kind="ExternalInput").ap()
    mask_ssq = nc.dram_tensor("mask_ssq", [128, FT, H], BF16,
                              kind="ExternalInput").ap()
    sel_q = nc.dram_tensor("sel_q", [H, C], BF16, kind="ExternalInput").ap()
    sel_k = nc.dram_tensor("sel_k", [H, C], BF16, kind="ExternalInput").ap()
    vinit = nc.dram_tensor("vinit", [128, H], BF16, kind="ExternalInput").ap()
    yT = nc.dram_tensor("yT", [B_local, C, N], BF16, kind="ExternalOutput").ap()
    if dbg:
        dbg_q = nc.dram_tensor("dbg_q", [C, N], BF16, kind="ExternalOutput").ap()
        dbg_k = nc.dram_tensor("dbg_k", [C, N], BF16, kind="ExternalOutput").ap()
        dbg_araw = nc.dram_tensor("dbg_araw", [C, N], BF16,
                                  kind="ExternalOutput").ap()
        dbg_invden = nc.dram_tensor("dbg_invden", [H, N], BF16,
                                    kind="ExternalOutput").ap()
        dbg_va = nc.dram_tensor("dbg_va", [128, H, 66], BF16,
                                kind="ExternalOutput").ap()
        dbg_dens = nc.dram_tensor("dbg_dens", [8, 2, 2, 512], F32,
                                  kind="ExternalOutput").ap()
        dbg_dens2 = nc.dram_tensor("dbg_dens2", [8, 2, 2, 512], F32,
                                   kind="ExternalOutput").ap()

    with tile.TileContext(nc) as tc, ExitStack() as ctx:
        singles = ctx.enter_context(tc.tile_pool(name="singles", bufs=1))
        xp = ctx.enter_context(tc.tile_pool(name="xp", bufs=1))
        wp = ctx.enter_context(tc.tile_pool(name="wp", bufs=2))
        qkp = ctx.enter_context(tc.tile_pool(name="qkp", bufs=2 * FT))
        sqp = ctx.enter_context(tc.tile_pool(name="sqp", bufs=1))
        vap = ctx.enter_context(tc.tile_pool(name="vap", bufs=2 * KT))
        ptp = ctx.enter_context(tc.tile_pool(name="ptp", bufs=2))
        aop = ctx.enter_context(tc.tile_pool(name="aop", bufs=2 * FT))
        statp = ctx.enter_context(tc.tile_pool(name="statp", bufs=2))
        ystp = ctx.enter_context(tc.tile_pool(name="ystp", bufs=2))

        mmps = ctx.enter_context(tc.tile_pool(name="mmps", bufs=2, space="PSUM"))
        stps = ctx.enter_context(tc.tile_pool(name="stps", bufs=2, space="PSUM"))
        avps = ctx.enter_context(tc.tile_pool(name="avps", bufs=2, space="PSUM"))

        # ---- constants / weights resident across items ----
        mask_sb = singles.tile([128, FT, H], BF16)
        nc.sync.dma_start(out=mask_sb, in_=mask_ssq)
        selq_sb = singles.tile([H, C], BF16)
        nc.sync.dma_start(out=selq_sb, in_=sel_q)
        selk_sb = singles.tile([H, C], BF16)
        nc.sync.dma_start(out=selk_sb, in_=sel_k)
        bias_sb = singles.tile([128, KT], F32)
        nc.sync.dma_start(out=bias_sb, in_=proj_b.rearrange("(k p) -> p k", p=128))
        eps_sb = singles.tile([H, 1], F32)
        nc.vector.memset(eps_sb, EPS)
        vw_sb = singles.tile([128, KT, C], BF16)
        nc.gpsimd.dma_start(out=vw_sb,
                            in_=v_wT.rearrange("(k p) f -> p k f", p=128))

        bsp = ctx.enter_context(tc.tile_pool(name="bsp", bufs=2))
        dsp = ctx.enter_context(tc.tile_pool(name="dsp", bufs=1))

        loop = ctx.enter_context(tc.For_i(0, reps, 1)) if reps > 1 else None
        for b in range(B_local):
            # ============ phase A: qkv projection + rmsnorm stats ==========
            x_sb = xp.tile([128, KT, N], BF16, tag="x", name=f"x_{b}")
            nc.gpsimd.dma_start(
                out=x_sb, in_=xT[b].rearrange("(k p) t -> p k t", p=128))

            ssq_q = statp.tile([H, N], F32, tag="ssqq")
            ssq_k = statp.tile([H, N], F32, tag="ssqk")
            q_sb, k_sb = {}, {}
            for ft in range(2 * FT):
                f = ft % FT
                wt = wp.tile([128, C], BF16, tag="qkw")
                nc.gpsimd.dma_start(out=wt, in_=qk_wT[ft])
                dstmap = q_sb if ft < FT else k_sb
                dst = qkp.tile([128, N], BF16, tag="qk",
                               name=f"{'q' if ft < FT else 'k'}_{b}_{f}")
                dstmap[f] = dst
                sqt = sqp.tile([128, N], BF16, tag="sq")
                acc = ssq_q if ft < FT else ssq_k
                for half in range(2):
                    tsl = slice(half * QC, (half + 1) * QC)
                    ps = mmps.tile([128, QC], F32, tag="mm")
                    for k in range(KT):
                        nc.tensor.matmul(ps, wt[:, k * 128:(k + 1) * 128],
                                         x_sb[:, k, tsl],
                                         start=(k == 0), stop=(k == KT - 1))
                    nc.vector.tensor_copy(dst[:, tsl], ps)
                    nc.vector.tensor_mul(sqt[:, tsl], dst[:, tsl], dst[:, tsl])
                    ps2 = mmps.tile([128, QC], F32, tag="mm")
                    nc.tensor.matmul(ps2[:H], mask_sb[:, f], sqt[:, tsl],
                                     start=True, stop=True)
                    if f == 0:
                        nc.vector.tensor_copy(acc[:, tsl], ps2[:H])
                    else:
                        nc.vector.tensor_add(acc[:, tsl], acc[:, tsl], ps2[:H])

            # rmsnorm: invr = 1/sqrt(ssq/Dh + eps), per tensor
            invr = {}
            for nm, acc in (("q", ssq_q), ("k", ssq_k)):
                nc.scalar.activation(out=acc, in_=acc, func=AF.Sqrt,
                                     bias=eps_sb, scale=1.0 / Dh)
                nc.vector.reciprocal_approx_fast(out=acc, in_=acc)
                ivr = statp.tile([H, N], BF16, tag="invr" + nm)
                nc.vector.tensor_copy(ivr, acc)
                invr[nm] = ivr

            # scale pass: q *= bcast(selq^T invr_q); k *= bcast(selk^T invr_k)
            for ft in range(2 * FT):
                f = ft % FT
                sel = selq_sb if ft < FT else selk_sb
                ivr = invr["q"] if ft < FT else invr["k"]
                dst = (q_sb if ft < FT else k_sb)[f]
                fsl = slice(f * 128, (f + 1) * 128)
                for half in range(2):
                    tsl = slice(half * QC, (half + 1) * QC)
                    bc = mmps.tile([128, QC], F32, tag="mm")
                    nc.tensor.matmul(bc, sel[:, fsl], ivr[:, tsl],
                                     start=True, stop=True)
                    nc.vector.tensor_mul(dst[:, tsl], dst[:, tsl], bc)

            if dbg and b == 0:
                for f in range(FT):
                    nc.sync.dma_start(out=dbg_q[f * 128:(f + 1) * 128, :],
                                      in_=q_sb[f])
                    nc.sync.dma_start(out=dbg_k[f * 128:(f + 1) * 128, :],
                                      in_=k_sb[f])

            # V projection: token-major augmented tiles [tok, head, Dh+1]
            va = {}
            for j in range(KT):
                vat = vap.tile([128, H, 66], BF16, tag="va", name=f"va_{b}_{j}")
                va[j] = vat
                nc.sync.dma_start(out=vat[:, :, 64:65], in_=vinit.unsqueeze(-1))
                for half in range(2):
                    vsl = slice(half * QC, (half + 1) * QC)
                    ps = mmps.tile([128, QC], F32, tag="mm")
                    for k in range(KT):
                        nc.tensor.matmul(ps, x_sb[:, k, j * 128:(j + 1) * 128],
                                         vw_sb[:, k, vsl],
                                         start=(k == 0), stop=(k == KT - 1))
                    nc.vector.tensor_copy(
                        vat[:, half * FT:(half + 1) * FT, 0:64],
                        ps.rearrange("p (h e) -> p h e", e=64))

            # ============ phase B: attention per head pair =================
            invden = statp.tile([H, N], BF16, tag="invden")
            den_all = statp.tile([H, N], F32, tag="denall")
            attn_sb = {}
            for p in range(NP):
                attn_sb[p] = aop.tile([128, N], BF16, tag="attn",
                                      name=f"attn_{b}_{p}")
            for p in range(NP):
                kA = k_sb[p][0:64]
                kB = k_sb[p][64:128]
                qA = q_sb[p][0:64]
                qB = q_sb[p][64:128]
                for qc in range(2):
                    qsl = slice(qc * QC, (qc + 1) * QC)
                    avA = avps.tile([65, QC], F32, tag="av")
                    avB = avps.tile([65, QC], F32, tag="av")
                    pts = {}
                    for j in range(KT + 1):
                        if j < KT:
                            jsl = slice(j * 128, (j + 1) * 128)
                            st = stps.tile([128, 2 * QC], F32, tag="st")
                            nc.tensor.matmul(st[:, 0:QC], kA[:, jsl],
                                             qA[:, qsl], start=True, stop=True)
                            nc.tensor.matmul(st[:, QC:2 * QC], kB[:, jsl],
                                             qB[:, qsl], start=True, stop=True)
                            pt = ptp.tile([128, 2 * QC], BF16, tag="pt")
                            nc.scalar.activation(out=pt, in_=st, func=AF.Exp)
                            pts[j] = pt
                        if j > 0:
                            pt1 = pts.pop(j - 1)
                            nc.tensor.matmul(avA, va[j - 1][:, 2 * p, 0:65],
                                             pt1[:, 0:QC],
                                             start=(j == 1), stop=(j == KT))
                            nc.tensor.matmul(avB, va[j - 1][:, 2 * p + 1, 0:65],
                                             pt1[:, QC:2 * QC],
                                             start=(j == 1), stop=(j == KT))
                    # head A features: partitions already aligned
                    nc.vector.tensor_copy(attn_sb[p][0:64, qsl], avA[0:64])
                    # head B features: partition-hop 0-63 -> 64-127 via DMA
                    bst = bsp.tile([64, QC], BF16, tag="bst")
                    nc.vector.tensor_copy(bst, avB[0:64])
                    nc.sync.dma_start(out=attn_sb[p][64:128, qsl], in_=bst)
                    # denominators: reciprocal at p64, tiny DMA into [H, N]
                    # stage raw dens in SBUF, gather to [H, N] via DMA;
                    # reciprocal runs once per item at partition base 0
                    dst_den = dsp.tile([128, 2, QC], F32, tag="denst")
                    nc.vector.tensor_copy(dst_den[64:65, 0], avA[64:65])
                    nc.vector.tensor_copy(dst_den[64:65, 1], avB[64:65])
                    nc.sync.dma_start(out=den_all[2 * p:2 * p + 1, qsl],
                                      in_=dst_den[64:65, 0])
                    nc.sync.dma_start(out=den_all[2 * p + 1:2 * p + 2, qsl],
                                      in_=dst_den[64:65, 1])
                    if dbg and b == 0:
                        nc.sync.dma_start(out=dbg_dens[p, qc].unsqueeze(0),
                                          in_=dst_den[64:65])
                        dact = dsp.tile([128, 2, QC], F32, tag="denact")
                        nc.scalar.activation(out=dact[64:65, 0],
                                             in_=avA[64:65], func=AF.Copy)
                        nc.scalar.activation(out=dact[64:65, 1],
                                             in_=avB[64:65], func=AF.Copy)
                        nc.sync.dma_start(out=dbg_dens2[p, qc].unsqueeze(0),
                                          in_=dact[64:65])

            if dbg and b == 0:
                nc.sync.dma_start(out=dbg_va, in_=va[0])
                nc.sync.dma_start(out=dbg_invden, in_=invden)
                for p in range(NP):
                    nc.sync.dma_start(out=dbg_araw[p * 128:(p + 1) * 128, :],
                                      in_=attn_sb[p])

            # invden = 1/den (reuse the dead ssq_q tile as f32 scratch)
            nc.vector.reciprocal_approx_fast(out=ssq_q, in_=den_all)
            nc.vector.tensor_copy(invden, ssq_q)

            # ============ phase C: normalize + output projection ===========
            for kf in range(FT):
                fsl = slice(kf * 128, (kf + 1) * 128)
                for half in range(2):
                    tsl = slice(half * QC, (half + 1) * QC)
                    bc = mmps.tile([128, QC], F32, tag="mm")
                    nc.tensor.matmul(bc, selk_sb[:, fsl], invden[:, tsl],
                                     start=True, stop=True)
                    nc.vector.tensor_mul(attn_sb[kf][:, tsl],
                                         attn_sb[kf][:, tsl], bc)
            for mt in range(FT):
                pw = wp.tile([128, C], BF16, tag="pw")
                nc.gpsimd.dma_start(out=pw, in_=proj_wT[mt])
                yst = ystp.tile([128, N], BF16, tag="yst")
                for half in range(2):
                    tsl = slice(half * QC, (half + 1) * QC)
                    ps = mmps.tile([128, QC], F32, tag="mm")
                    for k in range(KT):
                        nc.tensor.matmul(ps, pw[:, k * 128:(k + 1) * 128],
                                         attn_sb[k][:, tsl],
                                         start=(k == 0), stop=(k == KT - 1))
                    nc.vector.tensor_scalar_add(yst[:, tsl], ps,
                                                bias_sb[:, mt:mt + 1])
                nc.sync.dma_start(out=yT[b, mt * 128:(mt + 1) * 128, :],
                                  in_=yst)

    return nc


def prep_inputs(x, qkv_w, proj_w, proj_b, q_norm_w, k_norm_w, n_cores):
    """Host-side prep: shard over batch, pre-transpose, pre-cast bf16,
    build selector masks. Returns (in_maps, meta)."""
    import ml_dtypes

    BF = ml_dtypes.bfloat16
    B, N, C = x.shape
    H = C // 64
    Dh = 64
    B_local = B // n_cores
    FT = C // 128
    scale = Dh ** -0.5

    qkv_wT = np.ascontiguousarray(qkv_w.T)           # [C, 3C]
    qk_wT = np.ascontiguousarray(
        qkv_wT[:, :2 * C].reshape(C // 128, 128, 2 * C // 128, 128)
        .transpose(2, 1, 0, 3).reshape(2 * C // 128, 128, C)).astype(BF)
    v_wT = np.ascontiguousarray(qkv_wT[:, 2 * C:]).astype(BF)
    proj_wT = np.ascontiguousarray(
        proj_w.T.reshape(C // 128, 128, C // 128, 128)
        .transpose(2, 1, 0, 3).reshape(C // 128, 128, C)).astype(BF)

    # mask_sb[p, f, h] = 1 iff feature f*128+p belongs to head h (per tensor)
    feat = np.arange(C).reshape(FT, 128)             # [f, p]
    mask_ssq = np.ascontiguousarray(
        (feat.T[:, :, None] // Dh == np.arange(H)[None, None, :])
        .astype(BF))                                 # [128, FT, H]

    w_qk = (q_norm_w * k_norm_w).astype(np.float32)  # [Dh]
    heads = np.arange(C) // Dh
    sel_q = np.zeros((H, C), np.float32)
    sel_k = np.zeros((H, C), np.float32)
    for h in range(H):
        sel_q[h, h * Dh:(h + 1) * Dh] = scale * w_qk
        sel_k[h, h * Dh:(h + 1) * Dh] = 1.0

    shared = dict(qk_wT=qk_wT, v_wT=v_wT, proj_wT=proj_wT,
                  proj_b=proj_b.astype(np.float32), mask_ssq=mask_ssq,
                  sel_q=sel_q.astype(BF), sel_k=sel_k.astype(BF),
                  vinit=np.ones((128, H), BF))
    in_maps = []
    for i in range(n_cores):
        xs = x[i * B_local:(i + 1) * B_local]        # [B_local, N, C]
        xTl = np.ascontiguousarray(xs.transpose(0, 2, 1)).astype(BF)
        in_maps.append(dict(xT=xTl, **shared))
    return in_maps, dict(B=B, N=N, C=C, H=H, B_local=B_local)


def gather_output(results, meta):
    B, N, C, B_local = meta["B"], meta["N"], meta["C"], meta["B_local"]
    y = np.empty((B, N, C), np.float32)
    for i, r in enumerate(results):
        yTl = np.asarray(r["yT"], dtype=np.float32)  # [B_local, C, N]
        y[i * B_local:(i + 1) * B_local] = yTl.transpose(0, 2, 1)
    return y


N_CORES = 8
_CACHE = {}


def _get_nc():
    if "nc" not in _CACHE:
        from concourse import bacc

        nc = bacc.Bacc("TRN2", target_bir_lowering=False, debug=False,
                       num_devices=N_CORES)
        build_attention(nc, B_local=16 // N_CORES, N=1024, C=1024, H=16)
        nc.compile()
        _CACHE["nc"] = nc
    return _CACHE["nc"]


def run_sharded(in_maps, trace=False):
    from concourse.bass_utils import run_bass_kernel_spmd

    return run_bass_kernel_spmd(_get_nc(), in_maps,
                                core_ids=list(range(N_CORES)), trace=trace)


def kernel(x, qkv_w, proj_w, proj_b, q_norm_w, k_norm_w):
    in_maps, meta = prep_inputs(np.asarray(x), np.asarray(qkv_w),
                                np.asarray(proj_w), np.asarray(proj_b),
                                np.asarray(q_norm_w), np.asarray(k_norm_w),
                                N_CORES)
    res = run_sharded(in_maps)
    return gather_output(res.results, meta)
